# revision 25
# baseline (speedup 1.0000x reference)
"""AVWGCN (adaptive-embedding graph conv) Trainium2 Bass kernel.

Math (reference):
    A   = softmax(relu(E E^T), axis=1)            # [N, N], E: [N, D]
    T0  = I, T1 = A, T2 = 2 A A - I               # Chebyshev supports
    W   = einsum('nd,dkio->nkio', E, Wp)          # per-node weights
    b   = E @ bp                                  # per-node bias
    x_g = einsum('knm,bmc->bnkc', T, x)
    out = einsum('bnki,nkio->bno', x_g, W) + b

Restructuring (algebraically exact up to bf16 rounding):
  * Z := exp(relu(E E^T)) = max(exp(E E^T), 1) is SYMMETRIC; with row sums
    s, A = Z/s.  Aggregation matmuls use Z row-blocks as lhsT directly
    (lhsT.T @ rhs with symmetric Z) and fold 1/s into output scaling.
    Z row-blocks stream through 16 per-block DRAM tensors so phase C
    pipelines behind phase B.
  * y1 = A @ X,  u2' = 2 A y1  (so y2 = u2' - X);  the "- X" is folded
    into the weights: W_eff[k0] = Wp[k0] - Wp[k2] applied to x, Wp[k2]
    applied to u2'.
  * Final stage is node-parallel: the R matrix ([x^T; y1^T] and
    [u2'^T; ones], columns (b, n)) is exchanged via two AllToAll
    collectives so each core holds all 64 batches for its 256 nodes.
    Per-node weights W[n] = sum_d E[n,d] Wp_eff[d] (+ bias row, paired
    with the ones row of R) are built on the PE, then each node's output
    is two small accumulating matmuls — no d-expanded reduction at all.

Sharding: batch-parallel aggregation (8 cores x 8 batches), node-parallel
final stage (8 cores x 256 nodes) with AllToAll redistribution.
All matmul operands are bf16 (PSUM accumulation stays fp32).
"""

import os
import sys
import threading

sys.path.insert(0, "/opt/trn_rl_repo")

import numpy as np

import concourse.bass as bass  # noqa: E402
import concourse.mybir as mybir  # noqa: E402
from concourse import bacc  # noqa: E402
from concourse.tile import TileContext  # noqa: E402
from concourse.masks import make_identity  # noqa: E402
from concourse.bass_utils import run_bass_kernel_spmd  # noqa: E402

F32 = mybir.dt.float32
BF = mybir.dt.bfloat16
F16 = mybir.dt.float16
AF = mybir.ActivationFunctionType
OP = mybir.AluOpType

NCORES = 8
B = 64
BSH = B // NCORES          # 8 batches per core
N = 2048
C = 64                     # C_IN == C_OUT
D = 16                     # embedding dim
K = 3                      # Chebyshev order
P = 128                    # partitions
NT = N // P                # 16 node blocks
BC = BSH * C               # 512 = per-core (b, c) width
KI = K * C                 # 192 contraction for the final stage
NSH = N // NCORES          # 256 nodes per core in the final stage
KR = KI - P + 1            # 65: rows of the second R tile (y2 + ones)
HT = NT // 2               # 8 y1 accumulators per pass (PSUM banks)


def build_program():
    nc = bacc.Bacc("TRN2", target_bir_lowering=False, debug=False,
                   num_devices=NCORES)

    x_in = nc.dram_tensor("x", [BSH, N, C], F32, kind="ExternalInput")
    emb = nc.dram_tensor("emb", [N, D], F32, kind="ExternalInput")
    wp = nc.dram_tensor("wp", [D, K, C, C], F32, kind="ExternalInput")
    bp = nc.dram_tensor("bp", [D, C], F32, kind="ExternalInput")
    # node-sharded output: this core's NSH nodes, all B batches
    out_d = nc.dram_tensor("out", [B, NSH, C], F32, kind="ExternalOutput")
    # Z row-blocks as separate tensors => per-block DMA dependencies,
    # so phase C's row loads start as soon as each block is written.
    z_rows = [nc.dram_tensor(f"zd{m}", [P, N], BF) for m in range(NT)]
    GROUP = [list(range(NCORES))]

    with TileContext(nc) as tc:
        with tc.tile_pool(name="persist", bufs=1) as pp, \
             tc.tile_pool(name="ccd", bufs=1, space="DRAM") as ccd:
            ident = pp.tile([P, P], F32)
            make_identity(nc, ident[:])
            identb = pp.tile([P, P], BF)
            make_identity(nc, identb[:])

            # E node-major: [128, (t, d)] and transposed ET [16, 2048] bf16
            e_sb = pp.tile([P, NT * D], F32)
            for t in range(NT):
                nc.sync.dma_start(out=e_sb[:, t * D:(t + 1) * D],
                                  in_=emb[t * P:(t + 1) * P, :])
            et = pp.tile([D, N], BF)

            # Weight-pool staging for the per-node final stage.
            # wk01[d, o*128 + k*64 + i] = Wp_eff[d, k, i, o]  (k in {0,1})
            # wk2 [d, o*65 + i]         = Wp[d, 2, i, o];  i=64 col = bp[d, o]
            wk01 = pp.tile([D, P * C], BF)
            wk2 = pp.tile([D, (C + 1) * C], BF)
            with tc.tile_pool(name="wst", bufs=1) as wst:
                wpst = wst.tile([D, K * C * C], F32)
                for k in range(K):
                    nc.sync.dma_start(
                        out=wpst[:, k * C * C:(k + 1) * C * C],
                        in_=wp[:, k, :, :].rearrange("d i o -> d (i o)"))
                bpst = wst.tile([D, C], F32)
                nc.sync.dma_start(out=bpst[:], in_=bp[:, :])

                def kslice(k):
                    # view wpst k-slice as [d, o, i] (strided read)
                    return wpst[:, k * C * C:(k + 1) * C * C].rearrange(
                        "d (i o) -> d o i", o=C)

                def w01view(k):
                    return wk01[:].rearrange(
                        "d (o ki) -> d o ki", ki=P)[:, :, k * C:(k + 1) * C]

                # W_eff[k0] = Wp[k0] - Wp[k2] (folds the -x of T2 = 2AA - I)
                nc.vector.tensor_tensor(out=w01view(0), in0=kslice(0),
                                        in1=kslice(2), op=OP.subtract)
                nc.vector.tensor_copy(w01view(1), kslice(1))
                nc.vector.tensor_copy(
                    wk2[:].rearrange("d (o i) -> d o i", i=C + 1)[:, :, 0:C],
                    kslice(2))
                nc.vector.tensor_copy(
                    wk2[:].rearrange("d (o i) -> d o i", i=C + 1)[:, :,
                                                                 C:C + 1],
                    bpst[:].unsqueeze(2))

            s_all = pp.tile([P, 2 * NT], F32)
            s_sb = s_all[:, 0:NT]
            sinv = s_all[:, NT:2 * NT]
            et_r = pp.tile([D, NSH], BF)  # E^T slice for this core's nodes

            # ---- build ET via PE transpose (f32 in, cast on copy) ----
            with tc.tile_pool(name="pet", bufs=4, space="PSUM") as pet:
                for t in range(NT):
                    ptile = pet.tile([D, P], F32)
                    nc.tensor.transpose(ptile[:], e_sb[:, t * D:(t + 1) * D],
                                        ident[:])
                    nc.any.tensor_copy(et[:, t * P:(t + 1) * P], ptile[:])

            # DRAM bounce tensors for the collectives.
            esend = ccd.tile([NCORES, D, NSH], BF)
            erecv = ccd.tile([NCORES, D, NSH], BF)
            r0send = ccd.tile([NCORES, P, BSH * NSH], BF)
            r0recv = ccd.tile([NCORES, P, BSH * NSH], BF)
            r1send = ccd.tile([NCORES, KR, BSH * NSH], BF)
            r1recv = ccd.tile([NCORES, KR, BSH * NSH], BF)

            # Tiny AllToAll: every core sends chunk j = E^T for core j's
            # nodes; every recv chunk holds MY node range (use chunk 0).
            for j in range(NCORES):
                nc.sync.dma_start(out=esend[j, :, :],
                                  in_=et[:, j * NSH:(j + 1) * NSH])
            nc.gpsimd.collective_compute(
                "AllToAll", OP.bypass, replica_groups=GROUP,
                ins=[esend.opt()], outs=[erecv.opt()])
            nc.sync.dma_start(out=et_r[:], in_=erecv[0, :, :])

            # y1 lives in its own outermost pool so phase F's tiles never
            # overlap (and thus never anti-depend on) its columns.
            with tc.tile_pool(name="pya", bufs=1) as pya:
                y1 = pya.tile([P, NT * BC], BF)
                xs = pya.tile([P, NT * BC], BF)

                # ===== Phases B+C fused: Z row-blocks + y1 accumulation ==
                # Pass h covers y1 output blocks t in [h*8, h*8+8): for each
                # Z row-block m (built on the fly in pass 0, streamed from
                # DRAM in pass 1), accumulate u1[t] += Zrow_m[:, t]^T @ xs[m].
                with tc.tile_pool(name="xst", bufs=2) as xstp:
                    for m in range(NT):
                        xst = xstp.tile([P, BC], F32)
                        nc.sync.dma_start(
                            out=xst[:].rearrange("p (b c) -> p b c", c=C),
                            in_=x_in[:, m * P:(m + 1) * P, :].transpose(
                                [1, 0, 2]))
                        nc.scalar.activation(xs[:, m * BC:(m + 1) * BC],
                                             xst[:], AF.Copy)

                    # ---- Phase B: Z row-blocks -> per-block DRAM ----
                    with tc.tile_pool(name="zb", bufs=3) as zbp, \
                         tc.tile_pool(name="psz", bufs=4,
                                      space="PSUM") as psz:
                        for m in range(NT):
                            zz = zbp.tile([P, N], BF)
                            for j in range(4):
                                zt = psz.tile([P, 512], F32)
                                nc.tensor.matmul(
                                    zt[:], et[:, m * P:(m + 1) * P],
                                    et[:, j * 512:(j + 1) * 512],
                                    start=True, stop=True)
                                nc.scalar.activation(
                                    zz[:, j * 512:(j + 1) * 512],
                                    zt[:], AF.Exp)
                            nc.vector.tensor_scalar_max(zz[:], zz[:], 1.0)
                            nc.vector.tensor_reduce(
                                out=s_sb[:, m:m + 1], in_=zz[:],
                                axis=mybir.AxisListType.X, op=OP.add)
                            nc.sync.dma_start(out=z_rows[m][:, :], in_=zz[:])
                    nc.vector.reciprocal(sinv, s_sb)

                    # ---- Phase C: y1 in two row-streaming passes ----
                    with tc.tile_pool(name="zrow", bufs=3) as zrp:
                        for half in range(2):
                            ts = range(half * HT, (half + 1) * HT)
                            with tc.tile_pool(name="pagg", bufs=1,
                                              space="PSUM") as pagg:
                                u1 = {}
                                for t in ts:
                                    u1[t] = pagg.tile([P, BC], F32,
                                                      name=f"u1_{t}")
                                for m in range(NT):
                                    zz = zrp.tile([P, N], BF)
                                    nc.sync.dma_start(out=zz[:],
                                                      in_=z_rows[m][:, :])
                                    for t in ts:
                                        nc.tensor.matmul(
                                            u1[t][:],
                                            zz[:, t * P:(t + 1) * P],
                                            xs[:, m * BC:(m + 1) * BC],
                                            start=(m == 0),
                                            stop=(m == NT - 1))
                                for t in ts:
                                    nc.vector.tensor_scalar_mul(
                                        y1[:, t * BC:(t + 1) * BC],
                                        u1[t][:], sinv[:, t:t + 1])

                # srep: 2/s[n] replicated on all rows (for u2' col scaling)
                with tc.tile_pool(name="psr", bufs=1) as psr:
                    srep = psr.tile([P, N], F32)
                    srow = psr.tile([1, N], F32)
                    sinv2 = psr.tile([P, NT], F32)
                    nc.vector.tensor_scalar_mul(sinv2[:], sinv, 2.0)
                    with tc.tile_pool(name="pst", bufs=1) as pst, \
                         tc.tile_pool(name="psts", bufs=1,
                                      space="PSUM") as psts:
                        stp = psts.tile([D, P], F32)
                        nc.tensor.transpose(stp[:], sinv2[:], ident[:])
                        st_sb = pst.tile([D, P], F32)
                        nc.any.tensor_copy(st_sb[:], stp[:])
                        for t in range(D):
                            nc.sync.dma_start(
                                out=srow[0:1, t * P:(t + 1) * P],
                                in_=st_sb[t:t + 1, :])
                    nc.gpsimd.partition_broadcast(srep[:], srow[0:1, :])

                    with tc.tile_pool(name="rcp", bufs=1) as rcp:
                        rc0 = rcp.tile([P, BSH * N], BF)  # x^T | y1^T rows
                        # ---- x^T into rc0 rows 0..C (k0 slot) ----
                        with tc.tile_pool(name="ptx", bufs=2,
                                          space="PSUM") as ptxp:
                            for m in range(NT):
                                ptx = ptxp.tile([C, BSH * P], BF)
                                for b in range(BSH):
                                    nc.tensor.transpose(
                                        ptx[:, b * P:(b + 1) * P],
                                        xs[:, m * BC + b * C:
                                           m * BC + (b + 1) * C],
                                        identb[:])
                                nc.vector.tensor_copy(
                                    rc0[0:C, :].rearrange(
                                        "c (b n) -> c b n",
                                        b=BSH)[:, :, m * P:(m + 1) * P],
                                    ptx[:].rearrange("c (b n) -> c b n",
                                                     b=BSH))
                        # ---- y1^T into rc0 rows C..P (k1 slot) ----
                        with tc.tile_pool(name="pty", bufs=2,
                                          space="PSUM") as ptyp:
                            for m in range(NT):
                                pty = ptyp.tile([P, BSH * P], BF)
                                for b in range(BSH):
                                    nc.tensor.transpose(
                                        pty[C:P, b * P:(b + 1) * P],
                                        y1[:, m * BC + b * C:
                                           m * BC + (b + 1) * C],
                                        identb[:],
                                        tile_position=(0, C))
                                nc.vector.tensor_copy(
                                    rc0[C:P, :].rearrange(
                                        "c (b n) -> c b n",
                                        b=BSH)[:, :, m * P:(m + 1) * P],
                                    pty[C:P, :].rearrange("c (b n) -> c b n",
                                                          b=BSH))

                        # rc0 ready: exchange it while D2 computes u2'.
                        for j in range(NCORES):
                            nc.sync.dma_start(
                                out=r0send[j, :, :].rearrange(
                                    "p (b n) -> p b n", n=NSH),
                                in_=rc0[:, :].rearrange(
                                    "p (b n) -> p b n",
                                    n=N)[:, :, j * NSH:(j + 1) * NSH])
                        nc.gpsimd.collective_compute(
                            "AllToAll", OP.bypass, replica_groups=GROUP,
                            ins=[r0send.opt()], outs=[r0recv.opt()])

                    # ========== Phase D2: u2' = 2 (Z @ y1) / s ===========
                    with tc.tile_pool(name="rc1p", bufs=1) as rc1p:
                        rc1 = rc1p.tile([KR, BSH * N], BF)
                        nc.vector.memset(rc1[C:C + 1, :], 1.0)
                        with tc.tile_pool(name="zl", bufs=3) as zlp, \
                             tc.tile_pool(name="pu2", bufs=2,
                                          space="PSUM") as pu2, \
                             tc.tile_pool(name="y2p", bufs=2) as y2p:
                            for nq in range(4):
                                u2t = pu2.tile([P, 4 * 512], F32)
                                for m in range(NT):
                                    zl = zlp.tile([P, 512], BF)
                                    nc.sync.dma_start(
                                        out=zl[:],
                                        in_=z_rows[m][:, nq * 512:
                                                      (nq + 1) * 512])
                                    for bc in range(4):
                                        nc.tensor.matmul(
                                            u2t[:, bc * 512:(bc + 1) * 512],
                                            y1[:, m * BC + bc * P:
                                               m * BC + (bc + 1) * P],
                                            zl[:],
                                            start=(m == 0),
                                            stop=(m == NT - 1))
                                # y2 = u2t * (2/s[n]), one broadcast multiply
                                y2t = y2p.tile([P, 4 * 512], BF)
                                nc.vector.tensor_tensor(
                                    out=y2t[:].rearrange(
                                        "p (q n) -> p q n", q=4),
                                    in0=u2t[:].rearrange(
                                        "p (q n) -> p q n", q=4),
                                    in1=srep[:, nq * 512:(nq + 1) * 512]
                                    .unsqueeze(1).broadcast_to((P, 4, 512)),
                                    op=OP.mult)
                                for bc in range(4):
                                    for h in range(2):
                                        bb = 2 * bc + h
                                        dst = rc1[0:C, bb * N + nq * 512:
                                                  bb * N + (nq + 1) * 512]
                                        src = y2t[h * C:(h + 1) * C,
                                                  bc * 512:(bc + 1) * 512]
                                        if h == 0:
                                            nc.scalar.activation(dst, src,
                                                                 AF.Copy)
                                        else:
                                            nc.sync.dma_start(out=dst,
                                                              in_=src)

                        # rc1 ready: exchange it.
                        for j in range(NCORES):
                            nc.sync.dma_start(
                                out=r1send[j, :, :].rearrange(
                                    "p (b n) -> p b n", n=NSH),
                                in_=rc1[:, :].rearrange(
                                    "p (b n) -> p b n",
                                    n=N)[:, :, j * NSH:(j + 1) * NSH])
                        nc.gpsimd.collective_compute(
                            "AllToAll", OP.bypass, replica_groups=GROUP,
                            ins=[r1send.opt()], outs=[r1recv.opt()])

                # ========= Phase F: per-node final stage =================
                # Rr0/Rr1: R columns for ALL 64 batches x my NSH nodes,
                # cols = (b_global, n_local) = (src*8 + b_local, n).
                # W0[ki, o*NSH + n] = W_eff[n, ki, o] (ki = k0 i | k1 i)
                # W1[i,  o*NSH + n] = W[n, k2, i, o]; row 64 = bias[n, o].
                # Per node: 2 accumulating matmuls, lhsT = Rr[:, b-cols of n]
                # (stride NSH), rhs = W[:, o-cols of n] (stride NSH), packed
                # 2 nodes (partition halves) x 8 nodes per PSUM bank.
                with tc.tile_pool(name="poolF", bufs=1) as pf:
                    w0 = pf.tile([P, C * NSH], BF)
                    w1 = pf.tile([KR, C * NSH], BF)
                    rr0 = pf.tile([P, B * NSH], BF)
                    rr1 = pf.tile([KR, B * NSH], BF)

                    # W build first: needs only et_r + wk tiles, so it
                    # overlaps the rc1 exchange.
                    with tc.tile_pool(name="psW", bufs=4,
                                      space="PSUM") as psw, \
                         tc.tile_pool(name="psV", bufs=4,
                                      space="PSUM") as psv:
                        for op_ in range(C // 2):
                            pw = psw.tile([P, 2 * NSH], F32)
                            pv = psv.tile([KR, 2 * NSH], F32)
                            for h in range(2):
                                o = 2 * op_ + h
                                nc.tensor.matmul(
                                    pw[:, h * NSH:(h + 1) * NSH],
                                    wk01[:, o * P:(o + 1) * P],
                                    et_r[:], start=True, stop=True)
                                nc.tensor.matmul(
                                    pv[:, h * NSH:(h + 1) * NSH],
                                    wk2[:, o * (C + 1):(o + 1) * (C + 1)],
                                    et_r[:], start=True, stop=True)
                            nc.vector.tensor_copy(
                                w0[:, 2 * op_ * NSH:(2 * op_ + 2) * NSH],
                                pw[:])
                            nc.scalar.activation(
                                w1[:, 2 * op_ * NSH:(2 * op_ + 2) * NSH],
                                pv[:], AF.Copy)

                    for i in range(NCORES):
                        nc.sync.dma_start(
                            out=rr0[:, i * BSH * NSH:(i + 1) * BSH * NSH],
                            in_=r0recv[i, :, :])
                        nc.sync.dma_start(
                            out=rr1[:, i * BSH * NSH:(i + 1) * BSH * NSH],
                            in_=r1recv[i, :, :])

                    rr0v = rr0[:, :].rearrange("p (b n) -> p b n", n=NSH)
                    rr1v = rr1[:, :].rearrange("p (b n) -> p b n", n=NSH)
                    w0v = w0[:, :].rearrange("p (o n) -> p o n", n=NSH)
                    w1v = w1[:, :].rearrange("p (o n) -> p o n", n=NSH)
                    NG = NSH // 16  # groups: 16 nodes = 2 halves x 8 slots
                    with tc.tile_pool(name="psF", bufs=4,
                                      space="PSUM") as psf, \
                         tc.tile_pool(name="outp", bufs=2) as outp:
                        for g in range(NG):
                            pout = psf.tile([P, 512], F32)
                            for idx in range(16):
                                n = 16 * g + idx
                                s, j = idx % 2, idx // 2
                                osl = pout[C * s:C * (s + 1),
                                           j * C:(j + 1) * C]
                                nc.tensor.matmul(
                                    osl, rr0v[:, :, n], w0v[:, :, n],
                                    start=True, stop=False,
                                    tile_position=(0, C * s))
                                nc.tensor.matmul(
                                    osl, rr1v[:, :, n], w1v[:, :, n],
                                    start=False, stop=True,
                                    tile_position=(0, C * s))
                            outsb = outp.tile([P, 512], F32)
                            nc.any.tensor_copy(outsb[:], pout[:])
                            for s in range(2):
                                nc.sync.dma_start(
                                    out=out_d[:, g * 16:(g + 1) * 16, :]
                                    .rearrange("b (j s) c -> s b j c",
                                               s=2)[s:s + 1],
                                    in_=outsb[C * s:C * (s + 1), :]
                                    .rearrange("b (j c) -> b j c", c=C))

    nc.compile()
    return nc


_CACHE = {}
_LOCK = threading.Lock()


def _get_program():
    with _LOCK:
        if "nc" not in _CACHE:
            _CACHE["nc"] = build_program()
        return _CACHE["nc"]


def kernel(x, node_embeddings, weights_pool, bias_pool):
    x = np.ascontiguousarray(np.asarray(x, dtype=np.float32))
    emb = np.ascontiguousarray(np.asarray(node_embeddings, dtype=np.float32))
    wp = np.ascontiguousarray(np.asarray(weights_pool, dtype=np.float32))
    bp = np.ascontiguousarray(np.asarray(bias_pool, dtype=np.float32))

    nc = _get_program()
    core_ids = list(range(NCORES))
    in_maps = [
        {"x": x[i * BSH:(i + 1) * BSH], "emb": emb, "wp": wp, "bp": bp}
        for i in core_ids
    ]
    trace = os.environ.get("KERNEL_TRACE", "") == "1"
    res = run_bass_kernel_spmd(nc, in_maps, core_ids, trace=trace)
    if trace:
        kernel.last_exec_time_ns = res.exec_time_ns
        kernel.last_results = res
    out = np.concatenate([res.results[i]["out"] for i in core_ids], axis=1)
    return out


kernel.last_exec_time_ns = None

if __name__ == "__main__":
    rng = np.random.default_rng(0)
    ins = {
        "x": rng.standard_normal((B, N, C), dtype=np.float32),
        "node_embeddings": rng.standard_normal((N, D), dtype=np.float32),
        "weights_pool": (rng.standard_normal((D, K, C, C), dtype=np.float32)
                         * 0.1),
        "bias_pool": rng.standard_normal((D, C), dtype=np.float32) * 0.1,
    }
    out = kernel(**ins)
    print("out", out.shape, out.dtype, float(np.abs(out).mean()))
